# revision 18
# baseline (speedup 1.0000x reference)
"""Self-contained Trainium2 Bass kernel for nn_Attention_41472204210330.

Multi-head attention (B=2, T=2048, HIDDEN=1024, 16 heads, head_dim=64, fp32)
with RoPE, sharded over 8 NeuronCores: data-parallel over the batch (2) x
tensor-parallel over heads (4 groups of 4 heads).  Each core computes its
batch's q/k/v projections for its 4 heads, RoPE, attention, and a partial
output projection (its heads' slice of wo); the host sums the 4 partials per
batch element.

Layout strategy (everything stays in [feature, token] "transposed" layout on
chip so the softmax reduction lands on the matmul contraction axis):
  - host pre-transposes x -> xT [1024, 2048] and the weight slices, and
    pre-casts matmul inputs to bf16 (PE runs 1 cycle/row for bf16 vs the
    measured 2 cycles/row for fp32/fp32r; all accumulation stays fp32 in
    PSUM; end-to-end resid_var ~4e-5).
  - qT/kT [256, 2048] come straight out of the projection matmuls; RoPE is
    applied during PSUM eviction (rotate-half = partition shift via
    SBUF->SBUF DMA, sin sign pre-baked on host).
  - scores are computed as ST = K Q^T [k_tok, q_tok]; exp() is applied while
    evicting PSUM on the scalar engine (scale=1/sqrt(64) folded in).  No max
    subtraction is needed: scores ~ N(0,1), |s| < ~6.
  - the softmax denominator is obtained by augmenting V with a ones column:
    OT_aug [65, q] = V_aug^T @ P^T, row 64 = sum_k exp.
  - attention runs per (head, 1024-token q-half) so one OT accumulator is
    only 2 PSUM banks; all PSUM tiles come from a single shared 4-slot pool
    so the phases pipeline without pool-boundary barriers.
  - normalization: the denominator row is reshaped to [128,8] through a
    DRAM bounce so the reciprocal runs on all DVE lanes, then broadcast
    back across the 64 dim partitions with a stride-0 DRAM read.
  - output projection produces yT [1024, 2048] partials, summed on host.
"""

import sys

if "/opt/trn_rl_repo" not in sys.path:
    sys.path.insert(0, "/opt/trn_rl_repo")

import numpy as np

import bass_rust
import concourse.bass as bass
import concourse.mybir as mybir
import concourse.tile as tile

HIDDEN = 1024
NUM_HEADS = 16
D = 64  # head dim
B = 2
T = 2048
N_CORES = 8
HPC = NUM_HEADS // (N_CORES // B)  # heads per core = 4
HD = HPC * D  # per-core head dims = 256
P = 128
F32 = mybir.dt.float32
BF16 = mybir.dt.bfloat16


def _split_waits(nc):
    """The in-container walrus caps semaphore waits per instruction lower
    than bass_rust/Tile assume ("Too many sync wait commands").  Hoist all
    but one semaphore wait per instruction onto nop instructions inserted
    just before it in the same engine's program order (semantically
    identical: all waits still complete before the instruction runs)."""
    from concourse._compat import not_none

    def make_nop(engine, wait):
        nop = nc.engines[engine].nop(nofuse=True)
        nop.ins.sync_info = bass_rust.SyncInfo(on_wait=[wait], on_update=[])
        return nop.ins

    tail_bb = not_none(nc.cur_bb).bb
    plans = []
    for fn in nc.m.functions:
        for bb in fn.blocks:
            plan = {}
            for inst in bb.instructions:
                si = inst.sync_info
                waits = list(si.on_wait) if si and si.on_wait else []
                sem = [w for w in waits if w.sync_type == "semaphore"]
                if len(sem) > 1:
                    plan[inst.name] = sem[:-1]
            if plan:
                plans.append((bb, plan))
    created = {}
    n_tail_before = len(tail_bb.instructions)
    for bb, plan in plans:
        eng_of = {i.name: i.engine for i in bb.instructions}
        for iname, hoists in plan.items():
            created[iname] = [make_nop(eng_of[iname], w) for w in hoists]
    created_names = {n.name for nops in created.values() for n in nops}
    tail_insts = [i for i in tail_bb.instructions if i.name not in created_names]
    assert len(tail_insts) == n_tail_before
    tail_bb.instructions = tail_insts
    for bb, plan in plans:
        out = []
        for inst in bb.instructions:
            if inst.name in plan:
                hoisted = plan[inst.name]
                out.extend(created[inst.name])
                si = inst.sync_info
                si.on_wait = [w for w in si.on_wait if w not in hoisted]
            out.append(inst)
        bb.instructions = out


def build_kernel():
    nc = bass.Bass("TRN2", target_bir_lowering=False, debug=False)

    xT = nc.dram_tensor("xT", [HIDDEN, T], BF16, kind="ExternalInput")
    wq_t = nc.dram_tensor("wq_t", [HIDDEN, HD], BF16, kind="ExternalInput")
    wk_t = nc.dram_tensor("wk_t", [HIDDEN, HD], BF16, kind="ExternalInput")
    wv_t = nc.dram_tensor("wv_t", [HIDDEN, HD], BF16, kind="ExternalInput")
    wo_t = nc.dram_tensor("wo_t", [HD, HIDDEN], BF16, kind="ExternalInput")
    cos2 = nc.dram_tensor("cos2", [P, T], F32, kind="ExternalInput")
    sin2 = nc.dram_tensor("sin2", [P, T], F32, kind="ExternalInput")
    yT = nc.dram_tensor("yT", [HIDDEN, T], F32, kind="ExternalOutput")

    IC_CH = HIDDEN // P  # 8 input-channel chunks
    NKT = T // P  # 16 k tiles
    VW = D + 1  # v columns per head incl. ones column
    NQ = 1024  # tile free width (ACT op width / q-half)

    mm = nc.tensor.matmul

    with tile.TileContext(nc) as tc:
        with (
            nc.allow_low_precision(
                reason="bf16 matmul operands (fp32 PSUM accumulation); "
                "resid_var ~4e-5 end to end"
            ),
            tc.tile_pool(name="persist", bufs=1) as persist,
            tc.tile_pool(name="pt_pool", bufs=4) as pt_pool,
            tc.tile_pool(name="nrm_pool", bufs=2) as nrm_pool,
            tc.tile_pool(name="p1", bufs=2) as p1,
            tc.tile_pool(name="p3", bufs=4) as p3,
            # All PSUM comes from one shared pool (4 slots x 2 banks = all 8
            # banks) so consecutive phases pipeline instead of waiting for a
            # pool-scope boundary.
            tc.tile_pool(name="psum", bufs=4, space="PSUM") as psum,
            tc.tile_pool(name="dram_pool", bufs=4, space="DRAM") as dram_pool,
        ):
            # ---- persistent SBUF tensors --------------------------------
            qTr = [
                persist.tile([P, T], BF16, tag=f"qTr{m}", name=f"qTr{m}")
                for m in range(2)
            ]
            kTr = [
                persist.tile([P, T], BF16, tag=f"kTr{m}", name=f"kTr{m}")
                for m in range(2)
            ]
            # per k-tile V tiles: [:, h*65:h*65+64] = v dims for head h,
            # column h*65+64 = ones (softmax denominator trick).  Separate
            # tiles so attention only waits for the k-tiles it reads.
            v_sb = [
                persist.tile([P, HPC * VW], BF16, tag=f"v{kt}", name=f"v{kt}")
                for kt in range(NKT)
            ]
            otn = [
                persist.tile([P, T], BF16, tag=f"otn{m}", name=f"otn{m}")
                for m in range(2)
            ]
            wo_sb = persist.tile([P, 2, HIDDEN], BF16, tag="wo_sb", name="wo_sb")
            cos_sb = persist.tile([P, T], F32, tag="cos_sb", name="cos_sb")
            sin_sb = persist.tile([P, T], F32, tag="sin_sb", name="sin_sb")

            for kt in range(NKT):
                for h in range(HPC):
                    nc.vector.memset(
                        v_sb[kt][:, h * VW + D : h * VW + D + 1].bitcast(
                            mybir.dt.uint16
                        ),
                        0x3F80,  # bf16 bits of 1.0
                    )
            nc.sync.dma_start(
                out=wo_sb[:], in_=wo_t.rearrange("(c p) o -> p c o", p=P)
            )
            nc.sync.dma_start(out=cos_sb[:], in_=cos2[:, :])
            nc.sync.dma_start(out=sin_sb[:], in_=sin2[:, :])

            w_sbs = {}
            for name, w in (("q", wq_t), ("k", wk_t), ("v", wv_t)):
                w_sb = persist.tile(
                    [P, IC_CH, HD], BF16, tag=f"w_{name}", name=f"w_{name}"
                )
                nc.sync.dma_start(
                    out=w_sb[:], in_=w.rearrange("(c p) o -> p c o", p=P)
                )
                w_sbs[name] = w_sb
            x_sb = [
                persist.tile([P, T], BF16, tag=f"x{c}", name=f"x{c}")
                for c in range(IC_CH)
            ]
            for c in range(IC_CH):
                nc.sync.dma_start(
                    out=x_sb[c][:], in_=xT[c * P : (c + 1) * P, :]
                )

            # ---- phase 1: projections + RoPE -----------------------------
            rope_ctr = [0]

            def project_qk_gen(name, dst, m):
                '''Emits one head-pair projection + RoPE; yields after every
                couple of matmuls so the caller can interleave it into
                ACT-bound attention loops.'''
                w_sb = w_sbs[name]
                for n in range(T // NQ):
                    ts = n * NQ
                    cs = slice(ts, ts + NQ)
                    ps = psum.tile([P, NQ], F32, tag="main", name="ps")
                    for sub in range(NQ // 512):
                        ss = slice(sub * 512, (sub + 1) * 512)
                        xs = slice(ts + sub * 512, ts + (sub + 1) * 512)
                        for c in range(IC_CH):
                            mm(
                                ps[:, ss],
                                w_sb[:, c, m * P : (m + 1) * P],
                                x_sb[c][:, xs],
                                start=(c == 0),
                                stop=(c == IC_CH - 1),
                            )
                            if c % 2 == 1:
                                yield
                    # RoPE: out = q*cos + rotate_half(q)*sin
                    qsb = p1.tile([P, NQ], F32, tag="qsb", name="qsb")
                    nc.vector.tensor_copy(out=qsb[:], in_=ps[:])
                    rot = p1.tile([P, NQ], F32, tag="rot", name="rot")
                    for blk in range(4):
                        src = (blk ^ 1) * 32  # swap 32-row halves
                        nc.sync.dma_start(
                            out=rot[blk * 32 : blk * 32 + 32, :],
                            in_=qsb[src : src + 32, :],
                        )
                    nc.vector.tensor_mul(
                        out=dst[m][:, cs], in0=ps[:], in1=cos_sb[:, cs]
                    )
                    tmp = p1.tile([P, NQ], F32, tag="tmp", name="tmp")
                    # alternate the 2-input ops between GpSimd and DVE
                    eng = nc.gpsimd if rope_ctr[0] % 2 == 0 else nc.vector
                    rope_ctr[0] += 1
                    eng.tensor_mul(out=tmp[:], in0=rot[:], in1=sin_sb[:, cs])
                    eng.tensor_add(
                        out=dst[m][:, cs], in0=dst[m][:, cs], in1=tmp[:]
                    )
                    yield

            def project_qk(name, dst, m):
                for _ in project_qk_gen(name, dst, m):
                    pass

            def project_v():
                for kt in range(NKT):
                    psv = psum.tile([P, HD], F32, tag="main", name="psv")
                    for c in range(IC_CH):
                        mm(
                            psv[:],
                            x_sb[c][:, kt * P : (kt + 1) * P],
                            w_sbs["v"][:, c, :],
                            start=(c == 0),
                            stop=(c == IC_CH - 1),
                        )
                    for h in range(HPC):
                        nc.vector.tensor_copy(
                            out=v_sb[kt][:, h * VW : h * VW + D],
                            in_=psv[:, h * D : (h + 1) * D],
                        )

            # ---- phase 2: attention for one (head, q-half) ---------------
            def attend(h, hf, filler=None):
                m = h // 2
                r0 = (h % 2) * D
                qh = slice(hf * NQ, (hf + 1) * NQ)
                ot = psum.tile([D + 1, NQ], F32, tag="main", name="ot")
                for kt in range(NKT):
                    if filler is not None:
                        next(filler, None)
                    pt = pt_pool.tile([P, NQ], BF16, tag="pt", name="pt")
                    st = psum.tile([P, NQ], F32, tag="main", name="st")
                    for sub in range(NQ // 512):
                        q0 = hf * NQ + sub * 512
                        mm(
                            st[:, sub * 512 : (sub + 1) * 512],
                            kTr[m][r0 : r0 + D, kt * P : (kt + 1) * P],
                            qTr[m][r0 : r0 + D, q0 : q0 + 512],
                            start=True,
                            stop=True,
                        )
                    # exp((K Q^T)/sqrt(D)) while evicting PSUM
                    nc.scalar.activation(
                        out=pt[:],
                        in_=st[:],
                        func=mybir.ActivationFunctionType.Exp,
                        scale=float(1.0 / np.sqrt(D)),
                    )
                    for sub in range(NQ // 512):
                        ss = slice(sub * 512, (sub + 1) * 512)
                        mm(
                            ot[:, ss],
                            v_sb[kt][:, h * VW : (h + 1) * VW],
                            pt[:, ss],
                            start=(kt == 0),
                            stop=(kt == NKT - 1),
                        )
                # normalize rows 0..63 by row 64 (the exp sums): bounce the
                # den row through DRAM into [128,8] so reciprocal runs on all
                # DVE lanes, bounce back, broadcast-read across partitions.
                den_sb = nrm_pool.tile([1, NQ], F32, tag="den", name="den")
                nc.vector.tensor_copy(out=den_sb[:], in_=ot[D : D + 1, :])
                dden = dram_pool.tile([1, NQ], F32, tag="dden", name="dden")
                nc.sync.dma_start(out=dden[:], in_=den_sb[:])
                denp = nrm_pool.tile([P, NQ // P], F32, tag="denp", name="denp")
                nc.sync.dma_start(
                    out=denp[:], in_=dden.rearrange("o (p f) -> (o p) f", p=P)
                )
                nc.vector.reciprocal(out=denp[:], in_=denp[:])
                drec = dram_pool.tile([1, NQ], F32, tag="drec", name="drec")
                nc.sync.dma_start(
                    out=drec.rearrange("o (p f) -> (o p) f", p=P), in_=denp[:]
                )
                rb = nrm_pool.tile([D, NQ], F32, tag="rb", name="rb")
                src = drec[0:1, :]
                nc.sync.dma_start(
                    out=rb[:],
                    in_=bass.AP(
                        tensor=src.tensor,
                        offset=src.offset,
                        ap=[[0, D]] + [list(a) for a in src.ap[1:]],
                    ),
                )
                nc.vector.tensor_mul(
                    out=otn[m][r0 : r0 + D, qh], in0=ot[0:D, :], in1=rb[:]
                )

            # ---- phase 3: output projection ------------------------------
            def project_out_gen(n, use_act):
                '''Output projection for q-half n (one psum tile per yield).'''
                cs = slice(n * NQ, (n + 1) * NQ)
                for mo in range(HIDDEN // P):
                    ps = psum.tile([P, NQ], F32, tag="main", name="psy")
                    for sub in range(NQ // 512):
                        ss = slice(sub * 512, (sub + 1) * 512)
                        os_ = slice(
                            n * NQ + sub * 512, n * NQ + (sub + 1) * 512
                        )
                        for c in range(2):
                            mm(
                                ps[:, ss],
                                wo_sb[:, c, mo * P : (mo + 1) * P],
                                otn[c][:, os_],
                                start=(c == 0),
                                stop=(c == 1),
                            )
                        yield
                    ysb = p3.tile([P, NQ], F32, tag="ysb", name="ysb")
                    if use_act and mo % 2 == 1:
                        nc.scalar.copy(out=ysb[:], in_=ps[:])
                    else:
                        nc.vector.tensor_copy(out=ysb[:], in_=ps[:])
                    nc.sync.dma_start(
                        out=yT[mo * P : (mo + 1) * P, cs], in_=ysb[:]
                    )
                    yield

            # Emission schedule: head-pair 0 projections first so the
            # scalar-engine exp stream (the critical resource) starts early;
            # head-pair 1 projections and the first output-projection half
            # are drip-fed into the ACT-bound attention loops as fillers so
            # the PE never idles (keeps the HAM clock-gate warm too).
            import itertools

            project_qk("q", qTr, 0)
            project_qk("k", kTr, 0)
            project_v()
            filler_m1 = itertools.chain(
                project_qk_gen("q", qTr, 1), project_qk_gen("k", kTr, 1)
            )
            attend(0, 0, filler_m1)
            attend(0, 1, filler_m1)
            attend(1, 0, filler_m1)
            attend(1, 1, filler_m1)
            for _ in filler_m1:  # drain any remainder
                pass
            attend(2, 0)
            attend(3, 0)
            filler_o0 = project_out_gen(0, use_act=False)
            attend(2, 1, filler_o0)
            attend(3, 1, filler_o0)
            for _ in filler_o0:
                pass
            for _ in project_out_gen(1, use_act=True):
                pass
    _split_waits(nc)
    return nc


def _rope_tables():
    inv_freq = 1.0 / (10000.0 ** (np.arange(0, D, 2, dtype=np.float32) / D))
    t = np.arange(T, dtype=np.float32)
    freqs = t[:, None] * inv_freq[None, :]  # [T, 32]
    emb = np.concatenate((freqs, freqs), axis=-1)  # [T, 64]
    cos = np.cos(emb).T.astype(np.float32)  # [64, T]
    sin = np.sin(emb).T.astype(np.float32)
    sign = np.where(np.arange(D) < D // 2, -1.0, 1.0).astype(np.float32)
    sin_signed = sin * sign[:, None]
    cos2 = np.ascontiguousarray(np.concatenate([cos, cos], axis=0))  # [128,T]
    sin2 = np.ascontiguousarray(np.concatenate([sin_signed, sin_signed], 0))
    return cos2, sin2


def make_in_maps(x, wq, wk, wv, wo):
    import ml_dtypes

    bf = ml_dtypes.bfloat16
    cos2, sin2 = _rope_tables()
    in_maps = []
    for core in range(N_CORES):
        b, g = divmod(core, N_CORES // B)
        hs = slice(g * HD, (g + 1) * HD)
        in_maps.append(
            {
                "xT": np.ascontiguousarray(x[b].T).astype(bf),
                "wq_t": np.ascontiguousarray(wq[hs].T).astype(bf),
                "wk_t": np.ascontiguousarray(wk[hs].T).astype(bf),
                "wv_t": np.ascontiguousarray(wv[hs].T).astype(bf),
                "wo_t": np.ascontiguousarray(wo[:, hs].T).astype(bf),
                "cos2": cos2,
                "sin2": sin2,
            }
        )
    return in_maps


def gather_output(results):
    y = np.zeros((B, T, HIDDEN), dtype=np.float32)
    for core, res in enumerate(results):
        b = core // (N_CORES // B)
        y[b] += res["yT"].T
    return y


_NC = None


def kernel(x, wq, wk, wv, wo):
    global _NC
    from concourse.bass_utils import run_bass_kernel_spmd

    if _NC is None:
        _NC = build_kernel()
    in_maps = make_in_maps(
        np.asarray(x), np.asarray(wq), np.asarray(wk), np.asarray(wv), np.asarray(wo)
    )
    res = run_bass_kernel_spmd(_NC, in_maps, core_ids=list(range(N_CORES)))
    return gather_output(res.results)


# revision 19
# speedup vs baseline: 1.1052x; 1.1052x over previous
"""Self-contained Trainium2 Bass kernel for nn_Attention_41472204210330.

Multi-head attention (B=2, T=2048, HIDDEN=1024, 16 heads, head_dim=64, fp32)
with RoPE, sharded over 8 NeuronCores: data-parallel over the batch (2) x
tensor-parallel over heads (4 groups of 4 heads).  Each core computes its
batch's q/k/v projections for its 4 heads, RoPE, attention, and a partial
output projection (its heads' slice of wo); the host sums the 4 partials per
batch element.

Layout strategy (everything stays in [feature, token] "transposed" layout on
chip so the softmax reduction lands on the matmul contraction axis):
  - host pre-transposes x -> xT [1024, 2048] and the weight slices, and
    pre-casts matmul inputs to bf16 (PE runs 1 cycle/row for bf16 vs the
    measured 2 cycles/row for fp32/fp32r; all accumulation stays fp32 in
    PSUM; end-to-end resid_var ~4e-5).
  - qT/kT [256, 2048] come straight out of the projection matmuls; RoPE is
    applied during PSUM eviction (rotate-half = partition shift via
    SBUF->SBUF DMA, sin sign pre-baked on host).
  - scores are computed as ST = K Q^T [k_tok, q_tok]; exp() is applied while
    evicting PSUM on the scalar engine (scale=1/sqrt(64) folded in).  No max
    subtraction is needed: scores ~ N(0,1), |s| < ~6.
  - the softmax denominator is obtained by augmenting V with a ones column:
    OT_aug [65, q] = V_aug^T @ P^T, row 64 = sum_k exp.
  - attention runs per (head, 1024-token q-half) so one OT accumulator is
    only 2 PSUM banks; all PSUM tiles come from a single shared 4-slot pool
    so the phases pipeline without pool-boundary barriers.
  - normalization: the denominator row is reshaped to [128,8] through a
    DRAM bounce so the reciprocal runs on all DVE lanes, then broadcast
    back across the 64 dim partitions with a stride-0 DRAM read.
  - output projection produces yT [1024, 2048] partials, summed on host.
"""

import sys

if "/opt/trn_rl_repo" not in sys.path:
    sys.path.insert(0, "/opt/trn_rl_repo")

import numpy as np

import bass_rust
import concourse.bass as bass
import concourse.mybir as mybir
import concourse.tile as tile

HIDDEN = 1024
NUM_HEADS = 16
D = 64  # head dim
B = 2
T = 2048
N_CORES = 8
HPC = NUM_HEADS // (N_CORES // B)  # heads per core = 4
HD = HPC * D  # per-core head dims = 256
P = 128
F32 = mybir.dt.float32
BF16 = mybir.dt.bfloat16


def _split_waits(nc):
    """The in-container walrus caps semaphore waits per instruction lower
    than bass_rust/Tile assume ("Too many sync wait commands").  Hoist all
    but one semaphore wait per instruction onto nop instructions inserted
    just before it in the same engine's program order (semantically
    identical: all waits still complete before the instruction runs)."""
    from concourse._compat import not_none

    def make_nop(engine, wait):
        nop = nc.engines[engine].nop(nofuse=True)
        nop.ins.sync_info = bass_rust.SyncInfo(on_wait=[wait], on_update=[])
        return nop.ins

    tail_bb = not_none(nc.cur_bb).bb
    plans = []
    for fn in nc.m.functions:
        for bb in fn.blocks:
            plan = {}
            for inst in bb.instructions:
                si = inst.sync_info
                waits = list(si.on_wait) if si and si.on_wait else []
                sem = [w for w in waits if w.sync_type == "semaphore"]
                if len(sem) > 1:
                    plan[inst.name] = sem[:-1]
            if plan:
                plans.append((bb, plan))
    created = {}
    n_tail_before = len(tail_bb.instructions)
    for bb, plan in plans:
        eng_of = {i.name: i.engine for i in bb.instructions}
        for iname, hoists in plan.items():
            created[iname] = [make_nop(eng_of[iname], w) for w in hoists]
    created_names = {n.name for nops in created.values() for n in nops}
    tail_insts = [i for i in tail_bb.instructions if i.name not in created_names]
    assert len(tail_insts) == n_tail_before
    tail_bb.instructions = tail_insts
    for bb, plan in plans:
        out = []
        for inst in bb.instructions:
            if inst.name in plan:
                hoisted = plan[inst.name]
                out.extend(created[inst.name])
                si = inst.sync_info
                si.on_wait = [w for w in si.on_wait if w not in hoisted]
            out.append(inst)
        bb.instructions = out


def build_kernel():
    nc = bass.Bass("TRN2", target_bir_lowering=False, debug=False)

    xT = nc.dram_tensor("xT", [HIDDEN, T], BF16, kind="ExternalInput")
    wq_t = nc.dram_tensor("wq_t", [HIDDEN, HD], BF16, kind="ExternalInput")
    wk_t = nc.dram_tensor("wk_t", [HIDDEN, HD], BF16, kind="ExternalInput")
    wv_t = nc.dram_tensor("wv_t", [HIDDEN, HD], BF16, kind="ExternalInput")
    wo_t = nc.dram_tensor("wo_t", [HD, HIDDEN], BF16, kind="ExternalInput")
    cos2 = nc.dram_tensor("cos2", [P, T], F32, kind="ExternalInput")
    sin2 = nc.dram_tensor("sin2", [P, T], F32, kind="ExternalInput")
    yT = nc.dram_tensor("yT", [HIDDEN, T], F32, kind="ExternalOutput")

    IC_CH = HIDDEN // P  # 8 input-channel chunks
    NKT = T // P  # 16 k tiles
    VW = D + 1  # v columns per head incl. ones column
    NQ = 1024  # tile free width (ACT op width / q-half)

    mm = nc.tensor.matmul

    with tile.TileContext(nc) as tc:
        with (
            nc.allow_low_precision(
                reason="bf16 matmul operands (fp32 PSUM accumulation); "
                "resid_var ~4e-5 end to end"
            ),
            tc.tile_pool(name="persist", bufs=1) as persist,
            tc.tile_pool(name="pt_pool", bufs=4) as pt_pool,
            tc.tile_pool(name="nrm_pool", bufs=2) as nrm_pool,
            tc.tile_pool(name="p1", bufs=2) as p1,
            tc.tile_pool(name="p3", bufs=4) as p3,
            # All PSUM comes from one shared pool (4 slots x 2 banks = all 8
            # banks) so consecutive phases pipeline instead of waiting for a
            # pool-scope boundary.
            tc.tile_pool(name="psum", bufs=4, space="PSUM") as psum,
            tc.tile_pool(name="dram_pool", bufs=4, space="DRAM") as dram_pool,
        ):
            # ---- persistent SBUF tensors --------------------------------
            qTr = [
                persist.tile([P, T], BF16, tag=f"qTr{m}", name=f"qTr{m}")
                for m in range(2)
            ]
            kTr = [
                persist.tile([P, T], BF16, tag=f"kTr{m}", name=f"kTr{m}")
                for m in range(2)
            ]
            # per k-tile V tiles: [:, h*65:h*65+64] = v dims for head h,
            # column h*65+64 = ones (softmax denominator trick).  Separate
            # tiles so attention only waits for the k-tiles it reads.
            v_sb = [
                persist.tile([P, HPC * VW], BF16, tag=f"v{kt}", name=f"v{kt}")
                for kt in range(NKT)
            ]
            otn = [
                persist.tile([P, T], BF16, tag=f"otn{m}", name=f"otn{m}")
                for m in range(2)
            ]
            wo_sb = persist.tile([P, 2, HIDDEN], BF16, tag="wo_sb", name="wo_sb")
            cos_sb = persist.tile([P, T], F32, tag="cos_sb", name="cos_sb")
            sin_sb = persist.tile([P, T], F32, tag="sin_sb", name="sin_sb")

            # preload order matters: wq + x chunks go first on the sync
    # queues (they gate the very first matmul); everything else loads
            # in parallel through the gpsimd (SWDGE) queues.
            w_sbs = {}
            for name in ("q", "k", "v"):
                w_sbs[name] = persist.tile(
                    [P, IC_CH, HD], BF16, tag=f"w_{name}", name=f"w_{name}"
                )
            nc.sync.dma_start(
                out=w_sbs["q"][:], in_=wq_t.rearrange("(c p) o -> p c o", p=P)
            )
            x_sb = [
                persist.tile([P, T], BF16, tag=f"x{c}", name=f"x{c}")
                for c in range(IC_CH)
            ]
            for c in range(IC_CH):
                nc.sync.dma_start(
                    out=x_sb[c][:], in_=xT[c * P : (c + 1) * P, :]
                )
            for name, w in (("k", wk_t), ("v", wv_t)):
                nc.gpsimd.dma_start(
                    out=w_sbs[name][:], in_=w.rearrange("(c p) o -> p c o", p=P)
                )
            nc.gpsimd.dma_start(out=cos_sb[:], in_=cos2[:, :])
            nc.gpsimd.dma_start(out=sin_sb[:], in_=sin2[:, :])
            nc.gpsimd.dma_start(
                out=wo_sb[:], in_=wo_t.rearrange("(c p) o -> p c o", p=P)
            )
            for kt in range(NKT):
                for h in range(HPC):
                    nc.vector.memset(
                        v_sb[kt][:, h * VW + D : h * VW + D + 1].bitcast(
                            mybir.dt.uint16
                        ),
                        0x3F80,  # bf16 bits of 1.0
                    )

            # ---- phase 1: projections + RoPE -----------------------------
            rope_ctr = [0]

            def project_qk_gen(name, dst, m):
                '''Emits one head-pair projection + RoPE; yields after every
                couple of matmuls so the caller can interleave it into
                ACT-bound attention loops.'''
                w_sb = w_sbs[name]
                for n in range(T // NQ):
                    ts = n * NQ
                    cs = slice(ts, ts + NQ)
                    ps = psum.tile([P, NQ], F32, tag="main", name="ps")
                    for sub in range(NQ // 512):
                        ss = slice(sub * 512, (sub + 1) * 512)
                        xs = slice(ts + sub * 512, ts + (sub + 1) * 512)
                        for c in range(IC_CH):
                            mm(
                                ps[:, ss],
                                w_sb[:, c, m * P : (m + 1) * P],
                                x_sb[c][:, xs],
                                start=(c == 0),
                                stop=(c == IC_CH - 1),
                            )
                            if c % 2 == 1:
                                yield
                    # RoPE: out = q*cos + rotate_half(q)*sin
                    qsb = p1.tile([P, NQ], F32, tag="qsb", name="qsb")
                    nc.vector.tensor_copy(out=qsb[:], in_=ps[:])
                    rot = p1.tile([P, NQ], F32, tag="rot", name="rot")
                    for blk in range(4):
                        src = (blk ^ 1) * 32  # swap 32-row halves
                        nc.sync.dma_start(
                            out=rot[blk * 32 : blk * 32 + 32, :],
                            in_=qsb[src : src + 32, :],
                        )
                    nc.vector.tensor_mul(
                        out=dst[m][:, cs], in0=ps[:], in1=cos_sb[:, cs]
                    )
                    tmp = p1.tile([P, NQ], F32, tag="tmp", name="tmp")
                    # alternate the 2-input ops between GpSimd and DVE
                    eng = nc.gpsimd if rope_ctr[0] % 2 == 0 else nc.vector
                    rope_ctr[0] += 1
                    eng.tensor_mul(out=tmp[:], in0=rot[:], in1=sin_sb[:, cs])
                    eng.tensor_add(
                        out=dst[m][:, cs], in0=dst[m][:, cs], in1=tmp[:]
                    )
                    yield

            def project_qk(name, dst, m):
                for _ in project_qk_gen(name, dst, m):
                    pass

            def project_v_gen():
                for kt in range(NKT):
                    psv = psum.tile([P, HD], F32, tag="main", name="psv")
                    for c in range(IC_CH):
                        mm(
                            psv[:],
                            x_sb[c][:, kt * P : (kt + 1) * P],
                            w_sbs["v"][:, c, :],
                            start=(c == 0),
                            stop=(c == IC_CH - 1),
                        )
                    for h in range(HPC):
                        nc.vector.tensor_copy(
                            out=v_sb[kt][:, h * VW : h * VW + D],
                            in_=psv[:, h * D : (h + 1) * D],
                        )
                    yield

            # ---- phase 2: attention for one (head, q-half) ---------------
            def attend(h, hf, filler=None):
                m = h // 2
                r0 = (h % 2) * D
                qh = slice(hf * NQ, (hf + 1) * NQ)
                ot = psum.tile([D + 1, NQ], F32, tag="main", name="ot")
                for kt in range(NKT):
                    if filler is not None:
                        next(filler, None)
                    pt = pt_pool.tile([P, NQ], BF16, tag="pt", name="pt")
                    st = psum.tile([P, NQ], F32, tag="main", name="st")
                    for sub in range(NQ // 512):
                        q0 = hf * NQ + sub * 512
                        mm(
                            st[:, sub * 512 : (sub + 1) * 512],
                            kTr[m][r0 : r0 + D, kt * P : (kt + 1) * P],
                            qTr[m][r0 : r0 + D, q0 : q0 + 512],
                            start=True,
                            stop=True,
                        )
                    # exp((K Q^T)/sqrt(D)) while evicting PSUM
                    nc.scalar.activation(
                        out=pt[:],
                        in_=st[:],
                        func=mybir.ActivationFunctionType.Exp,
                        scale=float(1.0 / np.sqrt(D)),
                    )
                    for sub in range(NQ // 512):
                        ss = slice(sub * 512, (sub + 1) * 512)
                        mm(
                            ot[:, ss],
                            v_sb[kt][:, h * VW : (h + 1) * VW],
                            pt[:, ss],
                            start=(kt == 0),
                            stop=(kt == NKT - 1),
                        )
                # normalize rows 0..63 by row 64 (the exp sums): bounce the
                # den row through DRAM into [128,8] so reciprocal runs on all
                # DVE lanes, bounce back, broadcast-read across partitions.
                den_sb = nrm_pool.tile([1, NQ], F32, tag="den", name="den")
                nc.vector.tensor_copy(out=den_sb[:], in_=ot[D : D + 1, :])
                dden = dram_pool.tile([1, NQ], F32, tag="dden", name="dden")
                nc.sync.dma_start(out=dden[:], in_=den_sb[:])
                denp = nrm_pool.tile([P, NQ // P], F32, tag="denp", name="denp")
                nc.sync.dma_start(
                    out=denp[:], in_=dden.rearrange("o (p f) -> (o p) f", p=P)
                )
                nc.vector.reciprocal(out=denp[:], in_=denp[:])
                drec = dram_pool.tile([1, NQ], F32, tag="drec", name="drec")
                nc.sync.dma_start(
                    out=drec.rearrange("o (p f) -> (o p) f", p=P), in_=denp[:]
                )
                rb = nrm_pool.tile([D, NQ], F32, tag="rb", name="rb")
                src = drec[0:1, :]
                nc.sync.dma_start(
                    out=rb[:],
                    in_=bass.AP(
                        tensor=src.tensor,
                        offset=src.offset,
                        ap=[[0, D]] + [list(a) for a in src.ap[1:]],
                    ),
                )
                nc.vector.tensor_mul(
                    out=otn[m][r0 : r0 + D, qh], in0=ot[0:D, :], in1=rb[:]
                )

            # ---- phase 3: output projection ------------------------------
            def project_out_gen(n, use_act):
                '''Output projection for q-half n (one psum tile per yield).'''
                cs = slice(n * NQ, (n + 1) * NQ)
                for mo in range(HIDDEN // P):
                    ps = psum.tile([P, NQ], F32, tag="main", name="psy")
                    for sub in range(NQ // 512):
                        ss = slice(sub * 512, (sub + 1) * 512)
                        os_ = slice(
                            n * NQ + sub * 512, n * NQ + (sub + 1) * 512
                        )
                        for c in range(2):
                            mm(
                                ps[:, ss],
                                wo_sb[:, c, mo * P : (mo + 1) * P],
                                otn[c][:, os_],
                                start=(c == 0),
                                stop=(c == 1),
                            )
                        yield
                    ysb = p3.tile([P, NQ], F32, tag="ysb", name="ysb")
                    if use_act and mo % 2 == 1:
                        nc.scalar.copy(out=ysb[:], in_=ps[:])
                    else:
                        nc.vector.tensor_copy(out=ysb[:], in_=ps[:])
                    nc.sync.dma_start(
                        out=yT[mo * P : (mo + 1) * P, cs], in_=ysb[:]
                    )
                    yield

            # Emission schedule: head-pair 0 projections first so the
            # scalar-engine exp stream (the critical resource) starts early;
            # head-pair 1 projections and the first output-projection half
            # are drip-fed into the ACT-bound attention loops as fillers so
            # the PE never idles (keeps the HAM clock-gate warm too).
            import itertools

            project_qk("q", qTr, 0)
            project_qk("k", kTr, 0)
            v_gen = project_v_gen()
            next(v_gen)  # v[0] ready before attend(0,0)'s first attnv
            next(v_gen)  # v[1]
            attend(0, 0, v_gen)  # remaining 14 v k-tiles drip in JIT
            filler_m1 = itertools.chain(
                v_gen,
                project_qk_gen("q", qTr, 1),
                project_qk_gen("k", kTr, 1),
            )
            attend(0, 1, filler_m1)
            attend(1, 0, filler_m1)
            attend(1, 1, filler_m1)
            for _ in filler_m1:  # drain any remainder
                pass
            attend(2, 0)
            attend(3, 0)
            filler_o0 = project_out_gen(0, use_act=False)
            attend(2, 1, filler_o0)
            attend(3, 1, filler_o0)
            for _ in filler_o0:
                pass
            for _ in project_out_gen(1, use_act=True):
                pass
    _split_waits(nc)
    return nc


def _rope_tables():
    inv_freq = 1.0 / (10000.0 ** (np.arange(0, D, 2, dtype=np.float32) / D))
    t = np.arange(T, dtype=np.float32)
    freqs = t[:, None] * inv_freq[None, :]  # [T, 32]
    emb = np.concatenate((freqs, freqs), axis=-1)  # [T, 64]
    cos = np.cos(emb).T.astype(np.float32)  # [64, T]
    sin = np.sin(emb).T.astype(np.float32)
    sign = np.where(np.arange(D) < D // 2, -1.0, 1.0).astype(np.float32)
    sin_signed = sin * sign[:, None]
    cos2 = np.ascontiguousarray(np.concatenate([cos, cos], axis=0))  # [128,T]
    sin2 = np.ascontiguousarray(np.concatenate([sin_signed, sin_signed], 0))
    return cos2, sin2


def make_in_maps(x, wq, wk, wv, wo):
    import ml_dtypes

    bf = ml_dtypes.bfloat16
    cos2, sin2 = _rope_tables()
    in_maps = []
    for core in range(N_CORES):
        b, g = divmod(core, N_CORES // B)
        hs = slice(g * HD, (g + 1) * HD)
        in_maps.append(
            {
                "xT": np.ascontiguousarray(x[b].T).astype(bf),
                "wq_t": np.ascontiguousarray(wq[hs].T).astype(bf),
                "wk_t": np.ascontiguousarray(wk[hs].T).astype(bf),
                "wv_t": np.ascontiguousarray(wv[hs].T).astype(bf),
                "wo_t": np.ascontiguousarray(wo[:, hs].T).astype(bf),
                "cos2": cos2,
                "sin2": sin2,
            }
        )
    return in_maps


def gather_output(results):
    y = np.zeros((B, T, HIDDEN), dtype=np.float32)
    for core, res in enumerate(results):
        b = core // (N_CORES // B)
        y[b] += res["yT"].T
    return y


_NC = None


def kernel(x, wq, wk, wv, wo):
    global _NC
    from concourse.bass_utils import run_bass_kernel_spmd

    if _NC is None:
        _NC = build_kernel()
    in_maps = make_in_maps(
        np.asarray(x), np.asarray(wq), np.asarray(wk), np.asarray(wv), np.asarray(wo)
    )
    res = run_bass_kernel_spmd(_NC, in_maps, core_ids=list(range(N_CORES)))
    return gather_output(res.results)
